# revision 44
# baseline (speedup 1.0000x reference)
"""Masked dot-product attention on 8 Trainium2 NeuronCores, sparsity-aware.

Full inputs: queries/keys/values [16, 2048, 128] f32, valid_lens [16] int.
Reference semantics: rows q >= valid_len[b] are fully masked (-1e6 across all
keys), so softmax gives uniform weights and the output row is mean(V[b]).
Rows q < valid_len attend over ALL 2048 keys (no key-side masking).

Strategy:
  * Only q-tiles with at least one row q < valid_len need real attention:
    sum_b ceil(valid_len_b/128) tiles instead of 256. Fully-masked tiles are
    filled with mean(V[b]) on the host.
  * SPMD across 8 cores with load balance done via DATA PLACEMENT: one
    compiled program is a sequence of "phases" with capacities caps[p]
    (tiles), chosen by a runtime planner. Each (core, phase) slot is filled
    by the host with one contiguous run of q-tiles from a single batch plus
    that batch's K/V slab (duplicated into the input buffers as needed).
    Unused slot tiles are zero-padded (exp(0)=1 -> harmless, discarded).
  * All staging is done on host: qT = (Q*keep)^T fp16 (mask folded in, so
    masked rows' scores are 0 and exp gives uniform weights), kT = K^T fp16,
    vb = [V | 1] fp16 in [k-part, ktile, d+1] layout. The device does only:
      S^T[k,q] = sum_d kT[d,k] qT[d,q]            (PE, fp16)
      E = exp(S^T / sqrt(D))                      (ACT, fp16 out)
      P[q,d+1] = sum_k E[k,q] [V|1][k,d+1]        (PE, fp16)
      out[q,d] = P[q,d] / P[q,D]                  (DVE)
    The ACT engine (exp at 1 elem/lane/cycle) is the bottleneck; the program
    keeps it saturated by double-buffered S-chunk PSUM tiles and by
    interleaving PV matmuls of the previous q-block between S chunks.
  * Startup: a few tiny PE warmup matmuls start the PE DVFS boost early, and
    a single "boot" DMA lands exactly what the first S-chunk needs.
    Tail: the last (single-tile) block accumulates its PV inline with its
    exp chunks so almost nothing runs after the final ACTIVATE.
"""

import math
from contextlib import ExitStack

import numpy as np

import concourse.bacc as bacc
import concourse.bass as bass
import concourse.tile as tile
from concourse import mybir
from concourse.bass_utils import run_bass_kernel_spmd

B, Q, K, D = 16, 2048, 2048, 128
NCORES = 8
P = 128
NKT = K // P                 # 16 k-tiles
NQT = Q // P                 # 16 q-tiles per batch
KVW = K + NKT * (D + 1)      # kv slab width per partition: kT cols + vb cols
SCALE = 1.0 / math.sqrt(D)

F32 = mybir.dt.float32
F16 = mybir.dt.float16

# cost model (ns) for the planner
_PE_NS_PER_TILE = 2170.0          # (2048 + 2064) PE cycles @ ~1.9 GHz
_ACT_NS = lambda n: (n + 352.0) / 1.34   # one ACTIVATE over n elems/lane
_DMA_BNS = 1.0 / 358.0 * 1.25     # ns per byte incl. inefficiency margin
_DMA_FIXED = 600.0


def _blocks_of(cap):
    """Decompose a phase of `cap` tiles into q-block widths from {512,256,128}.

    Widths are powers of two so each S-matmul's PSUM output slice never
    straddles a 2KB PSUM bank boundary.
    """
    w = cap * P
    out = []
    for b in (512, 256, 128):
        while w >= b:
            out.append(b)
            w -= b
    return out


# ---------------------------------------------------------------- planner

def _partitions(total, max_part, max_len):
    """Non-increasing partitions of `total` into <=max_len parts <=max_part."""
    out = []

    def rec(rem, mx, cur):
        if rem == 0:
            out.append(tuple(cur))
            return
        if len(cur) == max_len:
            return
        for p in range(min(mx, rem), 0, -1):
            cur.append(p)
            rec(rem - p, p, cur)
            cur.pop()

    rec(total, max_part, [])
    return out


def _greedy_pack(nqt, caps):
    """Pack each batch's nqt tiles into slots (8 per capacity class).

    Returns per-class piece lists [(batch, t0, size), ...] or None.
    Rule: take the largest free cap <= remaining; if none, the smallest
    free cap >= remaining (final piece, slot partially padded).
    """
    avail = [(c, ci) for ci, c in enumerate(caps) for _ in range(8)]
    avail.sort()
    pieces = [[] for _ in caps]
    order = sorted(range(len(nqt)), key=lambda b: -nqt[b])
    for b in order:
        r = int(nqt[b])
        t0 = 0
        while r > 0:
            pick = None
            # largest cap <= r
            for i in range(len(avail) - 1, -1, -1):
                if avail[i][0] <= r:
                    pick = i
                    break
            if pick is None:
                # smallest cap >= r
                for i in range(len(avail)):
                    if avail[i][0] >= r:
                        pick = i
                        break
            if pick is None:
                return None
            cap, ci = avail.pop(pick)
            size = min(cap, r)
            pieces[ci].append((b, t0, size))
            t0 += size
            r -= size
    return pieces


def _est_cost(caps):
    nt = sum(caps)
    pe = nt * _PE_NS_PER_TILE
    act = 0.0
    for c in caps:
        for wb in _blocks_of(c):
            for cs in _chunks_of(wb):
                act += _ACT_NS(cs * wb)
    m = len(caps)
    dma_bytes = m * (KVW * P * 2) + nt * (P * P * 2) + nt * (P * D * 4)
    dma = dma_bytes * _DMA_BNS + (m + nt + 1) * _DMA_FIXED
    nblocks = sum(len(_blocks_of(c)) for c in caps)
    # a trailing single-tile phase lets the final PV drain inline (short tail)
    tail_pen = 0.0 if caps[-1] == 1 else 1500.0
    return max(pe, act, dma) + 150.0 * m + 200.0 * nblocks + tail_pen


def _plan(nqt):
    """Choose capacities + packing. Returns (caps, per-class pieces)."""
    T = int(np.sum(nqt))
    if T == 0:
        return None
    lb = (T + NCORES - 1) // NCORES
    best = None
    for nt in range(lb, lb + 9):
        for caps in _partitions(nt, 8, 6):
            pieces = _greedy_pack(nqt, caps)
            if pieces is None:
                continue
            c = _est_cost(caps)
            if best is None or c < best[0]:
                best = (c, caps, pieces)
    if best is None:
        caps = (8, 8, 8, 8)
        pieces = _greedy_pack(nqt, caps)
        best = (0.0, caps, pieces)
    return best[1], best[2]


# ------------------------------------------------------------ device code

def _chunks_of(wb):
    """k-tile chunk sizes per exp call: up to 1024 f32 psum elems (2 banks)."""
    c = max(1, 1024 // wb)
    out = []
    r = NKT
    while r > 0:
        out.append(min(c, r))
        r -= out[-1]
    return out


def _build_program(caps):
    nt = sum(caps)
    m = len(caps)
    nc = bacc.Bacc(name="attn_sp")

    kt_d = nc.dram_tensor("ktd", [m, P, K], F16, kind="ExternalInput")
    boot_d = nc.dram_tensor("bootd", [P, 4 * P + 512], F16, kind="ExternalInput")
    vb_d = nc.dram_tensor("vbd", [m, P, NKT * (D + 1)], F16, kind="ExternalInput")
    qt_d = nc.dram_tensor("qtd", [m, P, 8 * P], F16, kind="ExternalInput")
    # output is partition-major: [q-in-tile, slot*D + d]; host re-tiles
    out_d = nc.dram_tensor("out", [P, nt * D], F32, kind="ExternalOutput")

    with tile.TileContext(nc) as tc, ExitStack() as ctx:
        singles = ctx.enter_context(tc.tile_pool(name="singles", bufs=1))
        kvpool = ctx.enter_context(tc.tile_pool(name="kvpool", bufs=2))
        epool = ctx.enter_context(tc.tile_pool(name="epool", bufs=3))
        small = ctx.enter_context(tc.tile_pool(name="small", bufs=4))
        outp = ctx.enter_context(tc.tile_pool(name="outp", bufs=2))
        ps_s = ctx.enter_context(tc.tile_pool(name="ps_s", bufs=3, space="PSUM"))
        ps_pv = ctx.enter_context(tc.tile_pool(name="ps_pv", bufs=2, space="PSUM"))

        def load_phase(ph):
            kt_t = kvpool.tile([P, K], F16, tag="kt", bufs=2)
            nc.sync.dma_start(out=kt_t, in_=kt_d[ph])
            vb_t = kvpool.tile([P, NKT * (D + 1)], F16, tag="vb", bufs=3)
            nc.sync.dma_start(out=vb_t, in_=vb_d[ph])
            qt_t = kvpool.tile([P, 8 * P], F16, tag="qt", bufs=2)
            nc.sync.dma_start(out=qt_t, in_=qt_d[ph])
            return kt_t, vb_t, qt_t

        # PE p-state warmup: dummy matmuls on zeroed SBUF so the PE reaches
        # its boosted clock while the first input DMAs are in flight.
        warm = singles.tile([P, 2 * P], F16)
        nc.vector.memset(warm, 0.0)
        for _ in range(4):
            wps = ps_pv.tile([P, P], F32, tag="pv")
            nc.tensor.matmul(wps, lhsT=warm[:, 0:P], rhs=warm[:, P:])

        # startup: ONE boot DMA lands kt tiles 0-2 plus the first q-block
        # (one issue + one completion on the critical path), then the bulk
        # in need-ordered pieces.
        w0 = caps[0] * P
        wb0 = min(w0, 512)
        boot = singles.tile([P, 4 * P + 512], F16)
        nc.sync.dma_start(out=boot, in_=boot_d[:, :])
        kt0 = kvpool.tile([P, K], F16, tag="kt", bufs=2)
        nc.sync.dma_start(out=kt0[:, 4 * P : 8 * P], in_=kt_d[0][:, 4 * P : 8 * P])
        nc.sync.dma_start(out=kt0[:, 8 * P :], in_=kt_d[0][:, 8 * P :])
        qt0 = kvpool.tile([P, 8 * P], F16, tag="qt", bufs=2)
        vb0 = kvpool.tile([P, NKT * (D + 1)], F16, tag="vb", bufs=3)
        nc.sync.dma_start(out=vb0, in_=vb_d[0])
        if w0 > wb0:
            nc.sync.dma_start(out=qt0[:, wb0:w0], in_=qt_d[0][:, wb0:w0])

        # prev block state: [e_tile, vb_tile, base_slot, ntiles, o_grp, sync_out]
        prev = None
        drained = 0  # drained tiles of prev block

        def drain_one(j):
            e_prev, vb_prev, base_slot, ntile = prev[0], prev[1], prev[2], prev[3]
            pv = ps_pv.tile([P, D + 1], F32, tag="pv")
            for kt_i in range(NKT):
                nc.tensor.matmul(
                    pv,
                    lhsT=e_prev[:, kt_i, j * P : (j + 1) * P],
                    rhs=vb_prev[:, kt_i * (D + 1) : (kt_i + 1) * (D + 1)],
                    start=(kt_i == 0),
                    stop=(kt_i == NKT - 1),
                )
            if prev[4] is None:
                prev[4] = outp.tile([P, ntile * D], F32, tag="o", name="o_grp")
            o_grp = prev[4]
            recip = small.tile([P, 1], F32, tag="recip")
            nc.vector.reciprocal(recip, pv[:, D : D + 1])
            nc.vector.tensor_scalar_mul(
                o_grp[:, j * D : (j + 1) * D], in0=pv[:, 0:D], scalar1=recip
            )
            if j == ntile - 1:
                eng = nc.sync if prev[5] else nc.gpsimd
                eng.dma_start(
                    out=out_d[:, base_slot * D : (base_slot + ntile) * D],
                    in_=o_grp,
                )

        col = 0
        blocks_per_ph = [_blocks_of(c) for c in caps]
        for ph in range(m):
            kt_t, vb_t, qt_t = (kt0, vb0, qt0) if ph == 0 else load_phase(ph)
            colp = 0
            for bi, wb in enumerate(blocks_per_ph[ph]):
                ntile = wb // P
                csizes = _chunks_of(wb)
                nch = len(csizes)
                # final single-tile block: accumulate its PV chunk-by-chunk
                # so almost no PV work remains after the last ACT
                is_final = (
                    ph == m - 1
                    and bi == len(blocks_per_ph[ph]) - 1
                    and ntile == 1
                    and (prev is None or prev[3] == 1)
                )
                fpv = None
                e = epool.tile([P, NKT, wb], F16, tag="e")
                kt_base = 0
                for ch in range(nch):
                    cs = csizes[ch]
                    ps = ps_s.tile([P, cs, wb], F32, tag="ps", padded_shape=[P, 1024 // wb, wb])
                    boot0 = ph == 0 and bi == 0
                    for j in range(cs):
                        kt_i = kt_base + j
                        if ph == 0 and kt_i < 4:
                            lhsT = boot[:, kt_i * P : (kt_i + 1) * P]
                        else:
                            lhsT = kt_t[:, kt_i * P : (kt_i + 1) * P]
                        if boot0:
                            rhs = boot[:, 4 * P : 4 * P + wb]
                        else:
                            rhs = qt_t[:, colp : colp + wb]
                        nc.tensor.matmul(ps[:, j, :], lhsT=lhsT, rhs=rhs)
                    nc.scalar.activation(
                        out=e[:, kt_base : kt_base + cs, :],
                        in_=ps,
                        func=mybir.ActivationFunctionType.Exp,
                        scale=SCALE,
                    )
                    if prev is not None:
                        target = ((ch + 1) * prev[3]) // nch
                        while drained < target:
                            drain_one(drained)
                            drained += 1
                    if is_final:
                        if fpv is None:
                            fpv = ps_pv.tile([P, D + 1], F32, tag="pv")
                        for kt_i in range(kt_base, kt_base + cs):
                            nc.tensor.matmul(
                                fpv,
                                lhsT=e[:, kt_i, 0:P],
                                rhs=vb_t[:, kt_i * (D + 1) : (kt_i + 1) * (D + 1)],
                                start=(kt_i == 0),
                                stop=(kt_i == NKT - 1),
                            )
                    kt_base += cs
                if is_final:
                    recip = small.tile([P, 1], F32, tag="recip")
                    nc.vector.reciprocal(recip, fpv[:, D : D + 1])
                    o_fin = outp.tile([P, D], F32, tag="o")
                    nc.vector.tensor_scalar_mul(
                        o_fin, in0=fpv[:, 0:D], scalar1=recip
                    )
                    nc.sync.dma_start(
                        out=out_d[:, (col // P) * D : (col // P + 1) * D],
                        in_=o_fin,
                    )
                    prev = None
                else:
                    prev = [e, vb_t, col // P, ntile, None, ph >= m - 2]
                    drained = 0
                col += wb
                colp += wb
        if prev is not None:
            while drained < prev[3]:
                drain_one(drained)
                drained += 1
    nc.compile()
    return nc


_PROGRAMS = {}


def _get_nc(caps):
    caps = tuple(caps)
    if caps not in _PROGRAMS:
        _PROGRAMS[caps] = _build_program(caps)
    return _PROGRAMS[caps]


# -------------------------------------------------------------- host glue

def _prepare(queries, keys, values, valid_lens):
    queries = np.ascontiguousarray(np.asarray(queries, dtype=np.float32))
    keys = np.ascontiguousarray(np.asarray(keys, dtype=np.float32))
    values = np.ascontiguousarray(np.asarray(values, dtype=np.float32))
    vl = np.asarray(valid_lens).astype(np.int64)

    nqt = np.minimum((vl + P - 1) // P, NQT).astype(int)
    plan = _plan(nqt)

    # host fill for fully-masked tiles: uniform softmax over ALL keys
    meanv = values.mean(axis=1)  # [B, D] f32
    full = np.empty((B, Q, D), dtype=np.float32)
    for b in range(B):
        full[b, nqt[b] * P :, :] = meanv[b]

    if plan is None:
        return None, None, full

    caps, pieces = plan
    m = len(caps)
    nt = sum(caps)

    keep = (np.arange(Q, dtype=np.int64)[None, :] < vl[:, None])
    used = sorted({pc[0] for cls in pieces for pc in cls})
    KT16 = {}
    VB16 = {}
    QT16 = {}
    for b in used:
        KT16[b] = np.ascontiguousarray(keys[b].astype(np.float16).T)  # [D, K]
        vb = np.ones((P, NKT, D + 1), dtype=np.float16)
        vb[:, :, :D] = values[b].reshape(NKT, P, D).transpose(1, 0, 2)
        VB16[b] = vb.reshape(P, NKT * (D + 1))
        qm = queries[b] * keep[b][:, None]
        QT16[b] = np.ascontiguousarray(qm.astype(np.float16).T)  # [D, Q]

    in_maps = []
    scatter = []  # per core: list of (slot_tile_idx, batch, tile)
    wb0 = min(caps[0] * P, 512)
    for c in range(NCORES):
        boot_in = np.zeros((P, 4 * P + 512), dtype=np.float16)
        kt_in = np.zeros((m, P, K), dtype=np.float16)
        vb_in = np.zeros((m, P, NKT * (D + 1)), dtype=np.float16)
        qt_in = np.zeros((m, P, 8 * P), dtype=np.float16)
        sc = []
        base = 0
        for ci, cap in enumerate(caps):
            cls = pieces[ci]
            if c < len(cls):
                b, t0, size = cls[c]
                kt_in[ci] = KT16[b]
                vb_in[ci] = VB16[b]
                qt_in[ci, :, 0 : size * P] = QT16[b][:, t0 * P : (t0 + size) * P]
                if ci == 0:
                    boot_in[:, 0 : 4 * P] = KT16[b][:, 0 : 4 * P]
                    ww = min(size * P, wb0)
                    boot_in[:, 4 * P : 4 * P + ww] = QT16[b][
                        :, t0 * P : t0 * P + ww
                    ]
                for j in range(size):
                    sc.append((base + j, b, t0 + j))
            base += cap
        in_maps.append(
            {"ktd": kt_in, "vbd": vb_in, "qtd": qt_in, "bootd": boot_in}
        )
        scatter.append(sc)
    return (caps, in_maps, scatter), nqt, full


def _run(inputs: dict, trace: bool = False):
    plan, nqt, full = _prepare(**inputs)
    if plan is None:
        return full, None
    caps, in_maps, scatter = plan
    nc = _get_nc(caps)
    res = run_bass_kernel_spmd(
        nc, in_maps, core_ids=list(range(NCORES)), trace=trace
    )
    nt = sum(caps)
    for c in range(NCORES):
        out_c = res.results[c]["out"].reshape(P, nt, D).transpose(1, 0, 2)
        for slot, b, t in scatter[c]:
            full[b, t * P : (t + 1) * P, :] = out_c[slot]
    return full, res


def kernel(**inputs) -> np.ndarray:
    out, _ = _run(inputs, trace=False)
    return out
